# revision 22
# baseline (speedup 1.0000x reference)
"""BinaryConv (BN + sign-binarize + 3x3 binary conv) on 8 Trainium2 NeuronCores.

Strategy (data-parallel over batch, per sharding hint):
  - Each of the 8 cores gets 4 of the 32 images; weights/gamma/beta replicated.
  - Per-core BN partial stats (mean, mean-square per channel) via bn_stats,
    AllReduce'd across cores (tiny 2KB payload), then var/rstd/scale/bias.
  - Binarize activations with ScalarE Sign(scale*x+bias) into a zero-padded
    per-image layout (34-wide rows) in bf16.
  - 3x3 conv = 9 shifted matmuls per (ci-block, o-block) accumulated in PSUM.
    bf16 +/-1 inputs with fp32 PSUM accumulation are exact (integer sums).
  - Weights: sign() on ScalarE, transposed to [ci][o] via PE transpose.
"""

import numpy as np

import concourse.bass as bass
import concourse.tile as tile
from concourse import bacc, mybir
from concourse.bass_utils import run_bass_kernel_spmd
from concourse.masks import make_identity

F32 = mybir.dt.float32
BF16 = mybir.dt.bfloat16
FP8 = mybir.dt.float8e4

N_CORES = 8
N = 32            # full batch
NLOC = N // N_CORES  # images per core
C = 256           # channels (in == out)
HW = 32           # spatial
CB = C // 128     # ci partition blocks
OB = C // 128     # o partition blocks
EPS = 1e-5

PADW = HW + 2     # padded row width
IMG_PAD = 1160    # per-image padded buffer (>= 34*34 + 2 margin, mult of 8)
# output row-chunks (r0, r1): each chunk's matmul free dim = (r1-r0)*34 <= 512
CHUNKS = [(0, 11), (11, 22), (22, 32)]
TAPS = [(dy, dx) for dy in range(3) for dx in range(3)]


def _wt_idx(t: int, b: int, o: int) -> int:
    return (t * CB + b) * OB + o


def _build_body(ctx, nc, tc, x_d, g_d, be_d, w_d, y_d, cc_in, cc_out,
                dummy_in, dummy_out):
    # ---------------- pools ----------------
    const = ctx.enter_context(tc.tile_pool(name="const", bufs=1))
    xin_p = ctx.enter_context(tc.tile_pool(name="xin", bufs=1))
    wpool = ctx.enter_context(tc.tile_pool(name="wpool", bufs=1))
    apool = ctx.enter_context(tc.tile_pool(name="apool", bufs=1))
    stat_p = ctx.enter_context(tc.tile_pool(name="stat", bufs=1))
    out_p = ctx.enter_context(tc.tile_pool(name="outp", bufs=1))
    ps_tr = ctx.enter_context(tc.tile_pool(name="pstr", bufs=2, space="PSUM"))
    ps_acc = ctx.enter_context(tc.tile_pool(name="psacc", bufs=1, space="PSUM"))

    # ---------------- load x (stats-critical) ----------------
    xin = []
    for b in range(CB):
        xb = xin_p.tile([128, NLOC, HW, HW], F32, name=f"xin{b}", tag=f"xin{b}")
        for i in range(NLOC):
            nc.sync.dma_start(
                out=xb[:, i], in_=x_d[i, 128 * b : 128 * (b + 1), :, :]
            )
        xin.append(xb)

    # ---------------- weight prep (independent of stats) ----------------
    ident = const.tile([128, 128], BF16, name="ident")
    make_identity(nc, ident[:])



    # ---------------- zero only the padding of the activation buffers ------
    # (interior is fully overwritten by binarize; tiny strided memsets keep
    # both DVE and the collective-trigger path free)
    apad = [None] * NLOC
    for i in range(NLOC):
        ap = apool.tile([128, CB, IMG_PAD], FP8, name=f"apad{i}",
                        tag=f"apad{i}")
        nc.gpsimd.memset(ap[:, :, 0:35], 0.0)
        gaps = ap[:, :, 67 : 67 + 34 * HW].rearrange(
            "p b (h w) -> p b h w", w=PADW
        )[:, :, :, 0:2]
        nc.gpsimd.memset(gaps, 0.0)
        nc.gpsimd.memset(ap[:, :, 35 + 34 * HW - 2 : IMG_PAD], 0.0)
        apad[i] = ap

    # ---------------- local BN stats ----------------
    stats_rec = []
    for b in range(CB):
        xb = xin[b]
        rec = stat_p.tile([128, 2 * NLOC, 6], F32, name=f"rec{b}", tag=f"rec{b}")
        for i in range(NLOC):
            for h in range(2):
                nc.vector.bn_stats(
                    out=rec[:, 2 * i + h, :],
                    in_=xb[:, i, 16 * h : 16 * (h + 1), :].rearrange(
                        "p h w -> p (h w)"
                    ),
                )
        stats_rec.append(rec)

    # pack [mean_b, meansq_b] per ci-block into AllReduce payload
    arbuf = stat_p.tile([128, 2 * CB], F32, name="arbuf")
    tmp1 = stat_p.tile([128, 1], F32, name="tmp1")
    for b in range(CB):
        mv = stat_p.tile([128, 2], F32, name=f"mv{b}", tag=f"mv{b}")
        nc.vector.bn_aggr(out=mv[:], in_=stats_rec[b][:])
        nc.vector.tensor_copy(out=arbuf[:, 2 * b : 2 * b + 1], in_=mv[:, 0:1])
        nc.vector.tensor_mul(tmp1[:], mv[:, 0:1], mv[:, 0:1])
        nc.vector.tensor_add(arbuf[:, 2 * b + 1 : 2 * b + 2], mv[:, 1:2], tmp1[:])

    nc.gpsimd.dma_start(out=cc_in[:, :], in_=arbuf[:])
    nc.gpsimd.collective_compute(
        "AllReduce",
        mybir.AluOpType.add,
        replica_groups=[list(range(N_CORES))],
        ins=[cc_in.ap().opt()],
        outs=[cc_out.ap().opt()],
    )
    # ------- weight prep: traced after the AllReduce trigger so the 2.25MB
    # W transfer never queues ahead of the tiny stats bounce DMA; it has
    # ~40us of slack before the conv needs the prepared weights.
    wsign = []
    for o in range(OB):
        wraw = wpool.tile([128, C, 3, 3], F32, name=f"wraw{o}", tag=f"wraw{o}")
        nc.gpsimd.dma_start(
            out=wraw[:], in_=w_d[128 * o : 128 * (o + 1), :, :, :]
        )
        ws = wpool.tile([128, C, 3, 3], BF16, name=f"wsign{o}", tag=f"wsign{o}")
        nc.scalar.activation(
            out=ws[:], in_=wraw[:], func=mybir.ActivationFunctionType.Sign
        )
        wsign.append(ws)

    # transposed binarized weights, DoubleRow layout:
    # wT[ci_local, tap*OB+o, ci_half, o_local]  (fp8)
    wT = wpool.tile([128, len(TAPS) * OB, CB, 128], FP8, name="wT")
    for t, (dy, dx) in enumerate(TAPS):
        for b in range(CB):
            for o in range(OB):
                ptr = ps_tr.tile([128, 128], BF16, name="ptr", tag="ptr", bufs=2)
                nc.tensor.transpose(
                    ptr[:], wsign[o][:, 128 * b : 128 * (b + 1), dy, dx], ident[:]
                )
                nc.vector.tensor_copy(
                    out=wT[:, t * OB + o, b, :], in_=ptr[:]
                )

    # gs cols: [sum_mean_b0, sum_msq_b0, sum_mean_b1, sum_msq_b1] (x N_CORES)
    gs = stat_p.tile([128, 2 * CB], F32, name="gs")
    nc.gpsimd.dma_start(out=gs[:], in_=cc_out[:, :])
    smean = gs[:].rearrange("p (b s) -> p b s", s=2)[:, :, 0]  # [128, CB]
    smsq = gs[:].rearrange("p (b s) -> p b s", s=2)[:, :, 1]

    # per-channel scale/shift computed as wide [128, CB] ops.
    # Since std > 0:  sign((x-mean)*gamma/std + beta)
    #              == sign(gamma*x + (beta*std - mean*gamma))
    # so scale = gamma (known before the AllReduce!) and
    # shift = beta*std - mean*gamma  (no reciprocal needed).
    eps_t = const.tile([128, 1], F32, name="eps_t")
    nc.vector.memset(eps_t[:], EPS)
    gam = stat_p.tile([128, CB], F32, name="gam")
    bet = stat_p.tile([128, CB], F32, name="bet")
    for b in range(CB):
        nc.sync.dma_start(out=gam[:, b : b + 1], in_=g_d[128 * b : 128 * (b + 1), :])
        nc.sync.dma_start(out=bet[:, b : b + 1], in_=be_d[128 * b : 128 * (b + 1), :])
    inv = 1.0 / N_CORES
    # PE warm-up fodder: depends on the AllReduce result so the dummy matmuls
    # run exactly in the post-AllReduce latency window, un-throttling the HAM
    # clock gate right before the conv burst.
    junk = stat_p.tile([128, 4], BF16, name="junk")
    nc.scalar.copy(out=junk[:], in_=gs[:])
    mean_t = stat_p.tile([128, CB], F32, name="mean_t")
    nc.vector.tensor_scalar_mul(out=mean_t[:], in0=smean, scalar1=inv)
    msqr = stat_p.tile([128, CB], F32, name="msqr")
    nc.vector.tensor_mul(msqr[:], mean_t[:], mean_t[:])
    var_t = stat_p.tile([128, CB], F32, name="var_t")
    # var = (smsq * inv) - mean^2
    nc.vector.scalar_tensor_tensor(
        out=var_t[:],
        in0=smsq,
        scalar=inv,
        in1=msqr[:],
        op0=mybir.AluOpType.mult,
        op1=mybir.AluOpType.subtract,
    )
    std_t = stat_p.tile([128, CB], F32, name="std_t")
    nc.scalar.activation(
        out=std_t[:],
        in_=var_t[:],
        func=mybir.ActivationFunctionType.Sqrt,
        bias=eps_t[:],
        scale=1.0,
    )
    tmp_mg = stat_p.tile([128, CB], F32, name="tmp_mg")
    nc.vector.tensor_mul(tmp_mg[:], mean_t[:], gam[:])
    sh_t = stat_p.tile([128, CB], F32, name="sh_t")
    nc.vector.tensor_mul(sh_t[:], bet[:], std_t[:])
    nc.vector.tensor_sub(sh_t[:], sh_t[:], tmp_mg[:])
    scale_t = [gam[:, b : b + 1] for b in range(CB)]
    shift_t = [sh_t[:, b : b + 1] for b in range(CB)]

    # warm-up matmuls (results discarded)
    for k in range(12):
        pw = ps_tr.tile([128, 4], F32, name="pw", tag="ptr", bufs=2)
        nc.tensor.matmul(pw[:], ident[:], junk[:], start=True, stop=True)

    # ---------------- binarize into padded layout (fp8, DoubleRow pairs) ----
    for i in range(NLOC):
        for b in range(CB):
            interior = apad[i][:, b, 35 : 35 + 34 * HW].rearrange(
                "p (h w) -> p h w", w=PADW
            )[:, :, 0:HW]
            nc.scalar.activation(
                out=interior,
                in_=xin[b][:, i],
                func=mybir.ActivationFunctionType.Sign,
                scale=scale_t[b],
                bias=shift_t[b],
            )

    # ---------------- conv: 9 shifted DoubleRow matmuls, PSUM accumulate ----
    for i in range(NLOC):
        psum = {}
        for o in range(OB):
            for ci, (r0, r1) in enumerate(CHUNKS):
                psum[(o, ci)] = ps_acc.tile(
                    [128, (r1 - r0) * PADW], F32, name=f"acc{o}_{ci}",
                    tag=f"acc{o}_{ci}", bufs=1,
                )
        for t, (dy, dx) in enumerate(TAPS):
            toff = dy * PADW + dx
            first = t == 0
            last = t == len(TAPS) - 1
            for o in range(OB):
                lhsT = wT[:, t * OB + o, :, :]
                for ci, (r0, r1) in enumerate(CHUNKS):
                    ncols = (r1 - r0) * PADW
                    off = r0 * PADW + toff
                    nc.tensor.matmul(
                        psum[(o, ci)][:],
                        lhsT,
                        apad[i][:, :, off : off + ncols],
                        start=first,
                        stop=last,
                        perf_mode=mybir.MatmulPerfMode.DoubleRow,
                    )
        for o in range(OB):
            osb = out_p.tile([128, HW, HW], F32, name=f"osb{o}", tag=f"osb{o}",
                             bufs=2)
            for ci, (r0, r1) in enumerate(CHUNKS):
                nc.vector.tensor_copy(
                    out=osb[:, r0:r1, :],
                    in_=psum[(o, ci)][:].rearrange("p (r c) -> p r c", c=PADW)[
                        :, :, 0:HW
                    ],
                )
            nc.sync.dma_start(
                out=y_d[i, 128 * o : 128 * (o + 1), :, :], in_=osb[:]
            )


_CACHE: dict = {}


def _build():
    if "nc" in _CACHE:
        return _CACHE["nc"]
    nc = bacc.Bacc(
        "TRN2", target_bir_lowering=False, debug=False, num_devices=N_CORES
    )
    x_d = nc.dram_tensor("x", [NLOC, C, HW, HW], F32, kind="ExternalInput")
    g_d = nc.dram_tensor("gamma", [C, 1], F32, kind="ExternalInput")
    be_d = nc.dram_tensor("beta", [C, 1], F32, kind="ExternalInput")
    w_d = nc.dram_tensor("w", [C, C, 3, 3], F32, kind="ExternalInput")
    y_d = nc.dram_tensor("y", [NLOC, C, HW, HW], F32, kind="ExternalOutput")
    cc_in = nc.dram_tensor("cc_in", [128, 2 * CB], F32)
    cc_out = nc.dram_tensor("cc_out", [128, 2 * CB], F32, addr_space="Shared")
    dummy_in = nc.dram_tensor("dummy_in", [128, 1], F32)
    dummy_out = nc.dram_tensor(
        "dummy_out", [128, N_CORES], F32, addr_space="Shared"
    )

    from contextlib import ExitStack

    with tile.TileContext(nc) as tc, ExitStack() as ctx:
        _build_body(
            ctx, nc, tc, x_d, g_d, be_d, w_d, y_d, cc_in, cc_out,
            dummy_in, dummy_out,
        )
    nc.compile()
    _CACHE["nc"] = nc
    return nc


def kernel(x, gamma, beta, W):
    x = np.ascontiguousarray(np.asarray(x, dtype=np.float32))
    gamma = np.ascontiguousarray(np.asarray(gamma, dtype=np.float32)).reshape(C, 1)
    beta = np.ascontiguousarray(np.asarray(beta, dtype=np.float32)).reshape(C, 1)
    W = np.ascontiguousarray(np.asarray(W, dtype=np.float32))
    nc = _build()
    in_maps = [
        {
            "x": x[NLOC * k : NLOC * (k + 1)],
            "gamma": gamma,
            "beta": beta,
            "w": W,
        }
        for k in range(N_CORES)
    ]
    res = run_bass_kernel_spmd(nc, in_maps, core_ids=list(range(N_CORES)))
    return np.concatenate(
        [res.results[k]["y"] for k in range(N_CORES)], axis=0
    )
